# revision 17
# baseline (speedup 1.0000x reference)
"""OIM loss kernel for Trainium2, 8 NeuronCores, data-parallel over the roi dim.

Math (per reference):
    bank   = concat([lut, cq], 0)                      # [L=10532, D=256]
    logits = (inputs @ bank.T) * reliability * 30.0    # [N=8192, L]
    loss   = mean over rows with label != 5554 of
             logsumexp(logits[r]) - logits[r, label[r]]

Distribution: rows split 1024/core across 8 cores; the (reliability*30)-scaled
bank is replicated in fp8-e4m3 (inputs pre-scaled x16 so both fp8 operands sit
near unit variance; the exp folds 1/16 back in via its scale field).  Each
core returns [sum of masked nll, n_valid]; the host combines.

Per-core pipeline (v2 -- two concurrent PSUM consumers):
  PSUM is carved into a 4-deep ring of [128, 1024] quarters (2 banks each).
  PE streams fp8 DoubleRow matmuls into the ring; the exp work is split
  between TWO engines running concurrently on different quarters:
    - ACT units: table exp with fused accum_out (row-sum folded into the
      ACTIVATE; READ_ACCUMULATOR ~190ns amortized per unit).
    - DVE units: Schraudolph bit-exp (i16 = a*x+b viewed as bf16) into an
      SBUF slot, reduced downstream by a gpsimd add-tree (+ tiny DVE finish)
      or a DVE tensor_reduce, whichever engine is less loaded.
  A build-time greedy balancer assigns each of the 88 (row-tile x col-chunk)
  units to ACT or DVE so both engines stay ~100% busy; with 4 ring quarters
  each consumer ping-pongs between two quarters and PE refills are fully
  hidden.  ln(sumexp) via a bitcast log trick on DVE, picked logits as one
  fused dot, final cross-partition reduce on the PE.  Startup: bank/input
  DMAs fan out across 5 engine queues in ascending-column chunks so the
  first units start ~5us in; later chunks are dep-gated off unit anchors.
"""

import numpy as np
import ml_dtypes

N = 8192
D = 256
L = 10532  # 5532 + 5000
NCORES = 8
NSH = N // NCORES     # 1024 rows per core
P = 128               # partitions
RT = NSH // P         # 8 row tiles per core
KC = D // P           # 2 contraction chunks (DoubleRow pair)
IGNORE = 5554
OIM_SCALAR = 30.0
FP8_SCALE = 16.0      # inputs pre-scaled by this; exp() divides it back out

WQ = 1024             # PSUM ring quarter width (2 banks)
BANDS = 2             # full 4096-col bands; tail = 2340 = 1024+1024+292
TAILW = [1024, 1024, 292]
NB = 4 * BANDS + len(TAILW)   # units (blocksum slots) per row tile = 11

# Schraudolph exp on bf16 bit patterns: i16 = trunc(a*raw + b) viewed as bf16
# approximates exp(raw/16).  b tuned so block sums are unbiased under the
# truncating float->int convert.
SCHRAU_A = 128.0 / np.log(2.0) / FP8_SCALE
SCHRAU_B = 16249.136
# ln(x) ~= float_bits_as_int(x) * ln2/2^23 - C  (same trick in reverse)
FLN_K = float(np.log(2.0) / 2**23)
FLN_C = 88.02637566918142

BF16 = ml_dtypes.bfloat16
FP8 = ml_dtypes.float8_e4m3

NES = 10              # SBUF es slot ring depth for DVE bit-exp outputs

import os
VARIANT = os.environ.get("OIM_VARIANT", "full")  # full | allA | nogps
PICKED = os.environ.get("OIM_PICKED", "old")     # ttr | old | none
# ttr (tensor_tensor_reduce) passes CoreSim but dies on hardware -- do not use.

_CACHE = {}


def _unit_list():
    """All (rt, col_off, width) units in stream order (band-major)."""
    units = []
    for band in range(BANDS):
        for rt in range(RT):
            for qi in range(4):
                units.append((rt, band * 4096 + qi * WQ, WQ))
    for rt in range(RT):
        off = BANDS * 4096
        for w in TAILW:
            units.append((rt, off, w))
            off += w
    return units


def _plan(units):
    """Greedy consumer assignment balancing modeled ACT/DVE/gpsimd clocks.

    Returns per-unit ('A', None) or ('D', reducer) with reducer in
    {'gps', 'dve'}.  Models: ACT unit (w+352)/1.2 - 151 spacing overlap
    + 186 read-accumulator; DVE bit-exp (w+120)/0.96; DVE reduce
    (w+58)/0.96; gpsimd add-tree ~2.6 cyc/elem + dispatch.  gpsimd is a
    background reducer: it may trail the DVE exp stream by up to ~half
    the es slot ring (SLACK); the slack rule self-balances the gps/dve
    reducer mix to the capacity ratio.  No gps near the stream end so
    the tail isn't gated on a lagging gpsimd queue.
    """
    if VARIANT == "allA":
        return [("A", None)] * len(units)
    SLACK = 6000.0 if VARIANT != "nogps" else -1e18
    tA = tD = tG = 0.0
    plan = []
    for ui, (rt, off, w) in enumerate(units):
        if ui >= 16 and (ui - 16) % 4 == 0 and (ui - 16) // 4 < 8:
            tD += 390.0  # interspersed per-rt picked-logit dot on DVE
        cA = (w + 352) / 1.2 - 151 + 186
        cD = (w + 120) / 0.96
        cDR = (w + 58) / 0.96
        els = 0
        hw_ = w // 2
        steps = 0
        while hw_ >= 16:
            els += hw_
            hw_ //= 2
            steps += 1
        cG = (els * 2.6 + steps * 180) / 1.2
        cGfin = (16 + 58) / 0.96  # DVE finish of the gps tree
        can_gps = w == WQ and ui < len(units) - 8
        # reducer choice if this unit goes to DVE
        use_gps = can_gps and (max(tG, tD + cD) + cG) <= tD + cD + SLACK
        dcost = cD + (cGfin if use_gps else cDR)
        if tA + cA <= tD + dcost:
            plan.append(("A", None))
            tA += cA
        elif use_gps:
            plan.append(("D", "gps"))
            tG = max(tG, tD + cD) + cG
            tD += cD + cGfin
        else:
            plan.append(("D", "dve"))
            tD += cD + cDR
    return plan


def _build(debug=False):
    import concourse.bacc as bacc
    import concourse.tile as tile
    from concourse import mybir

    fp8 = mybir.dt.float8e4
    bf16 = mybir.dt.bfloat16
    f32 = mybir.dt.float32
    i16 = mybir.dt.int16
    i32 = mybir.dt.int32
    AF = mybir.ActivationFunctionType
    ALU = mybir.AluOpType
    AX = mybir.AxisListType
    DR = mybir.MatmulPerfMode.DoubleRow

    nc = bacc.Bacc(
        "TRN2", target_bir_lowering=False, debug=debug, enable_partition_id=False
    )

    # element (p, rt, k, c) = x16[rt*128 + c, k*128 + p]
    d_inp = nc.dram_tensor("inp", [P, RT, KC, P], fp8, kind="ExternalInput").ap()
    # element (p, k, j) = scaled[j, k*128 + p]
    d_bank = nc.dram_tensor("bank", [P, KC, L], fp8, kind="ExternalInput").ap()
    d_rows = nc.dram_tensor("rows", [P, RT, D], bf16, kind="ExternalInput").ap()
    d_bsel = nc.dram_tensor("bsel", [P, RT, D], bf16, kind="ExternalInput").ap()
    d_mask = nc.dram_tensor("mask", [P, RT], f32, kind="ExternalInput").ap()
    d_out = nc.dram_tensor("out", [1, 2], f32, kind="ExternalOutput").ap()

    units = _unit_list()
    plan = _plan(units)

    with tile.TileContext(nc) as tc:
        with (
            tc.tile_pool(name="const", bufs=1) as const,
            tc.tile_pool(name="psum", bufs=4, space="PSUM") as psum,
        ):
            # --- resident inputs ---
            inp_sb = const.tile([P, RT, KC, P], fp8)
            bank_sb = const.tile([P, KC, L], fp8)
            rows_sb = const.tile([P, RT, D], bf16)
            bsel_sb = const.tile([P, RT, D], bf16)
            mask_sb = const.tile([P, RT], f32)

            # --- startup DMA: ascending-column bank chunks on the 3 DMA-
            # capable queues (sync / scalar / gpsimd) ---
            nc.sync.dma_start(out=inp_sb[:, 0:2], in_=d_inp[:, 0:2])
            nc.sync.dma_start(
                out=bank_sb[:, :, 0:1024], in_=d_bank[:, :, 0:1024]
            )
            nc.scalar.dma_start(
                out=bank_sb[:, :, 1024:2560], in_=d_bank[:, :, 1024:2560]
            )
            nc.gpsimd.dma_start(
                out=bank_sb[:, :, 2560:4096], in_=d_bank[:, :, 2560:4096]
            )
            nc.sync.dma_start(out=inp_sb[:, 2:8], in_=d_inp[:, 2:8])
            # later chunks dep-gated off unit-consumer anchors.  All gated
            # DMAs ride the sync queue in ascending-anchor order: a gated
            # DMA on scalar/gpsimd would block that engine's in-order
            # instruction stream behind the anchor semaphore (deadlock).
            late_dmas = []  # (anchor unit idx, inst)
            late_dmas.append(
                (4, nc.sync.dma_start(
                    out=bank_sb[:, :, 4096:6144], in_=d_bank[:, :, 4096:6144]))
            )
            late_dmas.append((6, nc.sync.dma_start(out=rows_sb, in_=d_rows)))
            late_dmas.append(
                (8, nc.sync.dma_start(
                    out=bank_sb[:, :, 6144:8192], in_=d_bank[:, :, 6144:8192]))
            )
            late_dmas.append((10, nc.sync.dma_start(out=bsel_sb, in_=d_bsel)))
            late_dmas.append((12, nc.sync.dma_start(out=mask_sb, in_=d_mask)))
            late_dmas.append(
                (16, nc.sync.dma_start(
                    out=bank_sb[:, :, 8192:10532], in_=d_bank[:, :, 8192:10532]))
            )

            # --- ACT exp-table preload: tiny dummy exp scheduled first ---
            tiny = const.tile([P, 1], f32)
            nc.vector.memset(tiny, 0.0)
            tiny_o = const.tile([P, 1], f32)
            nc.scalar.activation(out=tiny_o, in_=tiny, func=AF.Exp)

            # --- PE warmup: ramp the HAM clock gate during the DMA wait ---
            wsrc = const.tile([P, KC, 512], fp8)
            nc.vector.memset(wsrc, 0.25)
            pw = psum.tile([P, WQ], f32, tag="ps", name="warm")
            warm_mms = []
            for i in range(6):
                m = nc.tensor.matmul(
                    pw[:, 0:512],
                    wsrc[:, :, 0:P],
                    wsrc,
                    start=True,
                    stop=True,
                    perf_mode=DR,
                )
                if warm_mms:
                    tile.add_dep_helper(m.ins, warm_mms[-1].ins, reason="warm order")
                warm_mms.append(m)

            # --- picked logit: per-rt fused dot (tensor_tensor_reduce),
            # interspersed in the DVE stream so no single op bubbles the
            # PSUM ring ---
            picked = const.tile([P, RT], f32)
            dots = const.tile([P, D], bf16)

            def emit_picked(rt):
                if PICKED == "none":
                    if rt == 0:
                        nc.vector.memset(picked, 0.0)
                    return
                if PICKED == "old":
                    nc.vector.tensor_mul(dots, rows_sb[:, rt], bsel_sb[:, rt])
                    nc.vector.tensor_reduce(
                        out=picked[:, rt : rt + 1], in_=dots,
                        axis=AX.X, op=ALU.add,
                    )
                    return
                nc.vector.tensor_tensor_reduce(
                    out=dots,
                    in0=rows_sb[:, rt],
                    in1=bsel_sb[:, rt],
                    scale=1.0,
                    scalar=0.0,
                    op0=ALU.mult,
                    op1=ALU.add,
                    accum_out=picked[:, rt : rt + 1],
                )

            def gps_tree(esl, w, acc):
                """Row-sum on gpsimd: in-place add tree w -> 16, DVE finishes."""
                hw_ = w // 2
                while hw_ >= 16:
                    nc.gpsimd.tensor_tensor(
                        esl[:, :hw_], esl[:, :hw_], esl[:, hw_ : 2 * hw_], op=ALU.add
                    )
                    hw_ //= 2
                nc.vector.tensor_reduce(
                    out=acc, in_=esl[:, :16], axis=AX.X, op=ALU.add
                )

            # --- main loop over units ---
            blocksums = const.tile([P, RT, NB], f32)
            es = const.tile([P, NES, WQ], bf16)
            # A-path exp target, never read; 2 rotating slots so the psum
            # WAR releases at ACTIVATE completion and consecutive A-exps
            # don't chain on a WAW drain
            trash = const.tile([P, 2, WQ], bf16)
            nta = 0
            nes = 0
            consumers = []  # per-unit psum-consumer instr, for DMA anchors
            slot_idx = [0] * RT
            for ui, ((rt, off, w), (eng, red)) in enumerate(zip(units, plan)):
                if ui >= 16 and (ui - 16) % 4 == 0 and (ui - 16) // 4 < RT:
                    emit_picked((ui - 16) // 4)
                ps = psum.tile([P, WQ], f32, tag="ps", name=f"ps_{ui}")
                lhsT = inp_sb[:, rt]
                nmm = (w + 511) // 512
                for b in range(nmm):
                    bw = min(512, w - b * 512)
                    nc.tensor.matmul(
                        ps[:, b * 512 : b * 512 + bw],
                        lhsT,
                        bank_sb[:, :, off + b * 512 : off + b * 512 + bw],
                        start=True,
                        stop=True,
                        perf_mode=DR,
                    )
                acc = blocksums[:, rt, slot_idx[rt] : slot_idx[rt] + 1]
                slot_idx[rt] += 1
                if eng == "A":
                    a = nc.scalar.activation(
                        out=trash[:, nta % 2, :w],
                        in_=ps[:, :w],
                        func=AF.Exp,
                        scale=1.0 / FP8_SCALE,
                        accum_out=acc,
                    )
                    nta += 1
                else:
                    esl = es[:, nes % NES]
                    nes += 1
                    a = nc.vector.tensor_scalar(
                        out=esl[:, :w].bitcast(i16),
                        in0=ps[:, :w],
                        scalar1=SCHRAU_A,
                        scalar2=SCHRAU_B,
                        op0=ALU.mult,
                        op1=ALU.add,
                    )
                    if red == "gps":
                        gps_tree(esl, w, acc)
                    else:
                        nc.vector.tensor_reduce(
                            out=acc, in_=esl[:, :w], axis=AX.X, op=ALU.add
                        )
                consumers.append(a)
            for anchor, dma in late_dmas:
                tile.add_dep_helper(
                    dma.ins,
                    consumers[anchor].ins,
                    reason="hold non-critical DMAs off the startup window",
                )

            # --- tail: nll = ln(sumexp) - picked, masked sums (no ACT) ---
            sumexp = const.tile([P, RT], f32)
            nc.vector.tensor_reduce(
                out=sumexp, in_=blocksums, axis=AX.X, op=ALU.add
            )
            lnse = const.tile([P, RT], f32)
            nc.vector.tensor_scalar(
                out=lnse,
                in0=sumexp.bitcast(i32),
                scalar1=FLN_K,
                scalar2=FLN_C,
                op0=ALU.mult,
                op1=ALU.subtract,
            )
            nll = const.tile([P, RT], f32)
            nc.vector.tensor_sub(nll, lnse, picked)
            masked = const.tile([P, RT], f32)
            nc.vector.tensor_mul(masked, nll, mask_sb)

            stacked = const.tile([P, 2], f32)
            nc.vector.tensor_reduce(
                out=stacked[:, 0:1], in_=masked, axis=AX.X, op=ALU.add
            )
            nc.vector.tensor_reduce(
                out=stacked[:, 1:2], in_=mask_sb, axis=AX.X, op=ALU.add
            )

            ones = const.tile([P, 1], f32)
            nc.vector.memset(ones, 1.0)
            fin = psum.tile([P, WQ], f32, tag="ps", name="fin")
            nc.tensor.matmul(fin[0:1, 0:2], ones, stacked, start=True, stop=True)
            out_sb = const.tile([1, 2], f32)
            nc.vector.tensor_copy(out=out_sb, in_=fin[0:1, 0:2])
            nc.sync.dma_start(out=d_out, in_=out_sb)

    nc.compile()
    return nc


def get_nc(debug=False):
    key = ("nc", debug)
    if key not in _CACHE:
        _CACHE[key] = _build(debug=debug)
    return _CACHE[key]


def make_in_maps(inputs, label, ious, lut, cq, reliability):
    """Host-side shard prep. Index gathers / transposes / casts only."""
    inputs = np.asarray(inputs, dtype=np.float32)
    label = np.asarray(label).astype(np.int64)
    lut = np.asarray(lut, dtype=np.float32)
    cq = np.asarray(cq, dtype=np.float32)
    reliability = np.asarray(reliability, dtype=np.float32)

    bank = np.concatenate([lut, cq], axis=0)                 # [L, D]
    scaled = bank * (OIM_SCALAR * reliability)[:, None]      # [L, D] fp32
    # [P, KC, L] fp8: (p, k, j) = scaled[j, k*128+p]
    bank8 = np.ascontiguousarray(
        scaled.T.reshape(KC, P, L).transpose(1, 0, 2)
    ).astype(FP8)

    valid = label != IGNORE
    safe = np.where(valid, label, 0)
    bsel_full = scaled[safe].astype(BF16)                    # [N, D]
    inp_bf = inputs.astype(BF16)                             # [N, D]
    inp8_full = (inputs * FP8_SCALE).astype(FP8)             # [N, D]

    in_maps = []
    for c in range(NCORES):
        sl = slice(c * NSH, (c + 1) * NSH)
        # [P, RT, KC, P]: (p, rt, k, c) = x16[rt*128+c, k*128+p]
        x8 = inp8_full[sl].astype(FP8)
        inp = np.ascontiguousarray(
            x8.T.reshape(KC, P, RT, P).transpose(1, 2, 0, 3)
        )
        x = inp_bf[sl]
        rows = np.ascontiguousarray(x.reshape(RT, P, D).transpose(1, 0, 2))
        bsel = np.ascontiguousarray(
            bsel_full[sl].reshape(RT, P, D).transpose(1, 0, 2)
        )
        mask = np.ascontiguousarray(
            valid[sl].reshape(RT, P).T.astype(np.float32)
        )
        in_maps.append(
            {"inp": inp, "bank": bank8, "rows": rows, "bsel": bsel, "mask": mask}
        )
    return in_maps


def _combine(parts):
    """parts: list of [1,2] arrays per core -> scalar loss."""
    arr = np.stack([np.asarray(p, dtype=np.float64) for p in parts])  # [8,1,2]
    total = arr[:, 0, 0].sum()
    count = arr[:, 0, 1].sum()
    return np.float32(total / max(count, 1.0))


def kernel(inputs, label, ious, lut, cq, reliability):
    from concourse import bass_utils

    nc = get_nc()
    in_maps = make_in_maps(inputs, label, ious, lut, cq, reliability)
    res = bass_utils.run_bass_kernel_spmd(nc, in_maps, core_ids=list(range(NCORES)))
    return _combine([r["out"] for r in res.results])
